# revision 36
# baseline (speedup 1.0000x reference)
"""Trainium2 Bass kernel for 12-head MHA (B=4, S=2048, D=768), 8 NeuronCores.

Sharding: core c -> (batch b = c//2, head-group g = c%2 of 6 heads).
Each core computes its batch's attention for its 6 heads plus the partial
out-projection; the host sums the two partial outputs per batch and adds b_out.

Device dataflow keeps the sequence axis on the SBUF free dimension everywhere,
so no on-chip transposes are needed:
  K^T / Q^T  : stationary = W columns (head-pair packed), moving = x^T chunks
  V          : stationary = x^T chunks, moving = W_v columns (natural layout)
  scores^T   : stationary = K^T tile, moving = Q^T cols (two heads row-tiled)
  exp        : ScalarE from PSUM, one [128, 2, 512] instruction per key tile
  attnV+den  : stationary = [V_h | ones] (M=65), moving = exp'd probs
  out proj   : stationary = pair-stacked context^T, moving = W_out rows

Matmuls run in float32r (TF32-class, 1 cycle/row at N>=256); ATTN_DT can drop
the attnV pair (probs, V) to bf16 for more speed at ~5x the error.

The schedule is software-pipelined: K/V projections first, then per q-chunk
Q projection -> attention -> normalization -> partial out-projection, so
TensorE and ScalarE overlap across stages.
"""

import sys

sys.path.insert(0, "/opt/trn_rl_repo")

from contextlib import ExitStack

import numpy as np

import concourse.bacc as bacc
import concourse.bass as bass
import concourse.tile as tile
from concourse import mybir
from concourse.bass_utils import run_bass_kernel_spmd

# If BASS_TRACE is set in the environment, run_bass_kernel_spmd imports
# antenv.axon_hooks, which is absent in this image. Register a stub so the
# call degrades to "no trace" instead of crashing.
try:
    import antenv.axon_hooks  # noqa: F401
except ImportError:
    import types as _types

    _stub = _types.ModuleType("antenv.axon_hooks")
    _stub.get_axon_ntff_profile_hook = lambda: None
    _stub.set_axon_ntff_profile_hook = lambda h: None
    sys.modules["antenv.axon_hooks"] = _stub

F32 = mybir.dt.float32
BF16 = mybir.dt.bfloat16
AF = mybir.ActivationFunctionType

MM_DT = mybir.dt.float32r   # dtype for projection / scores / out-proj matmuls
ATTN_DT = mybir.dt.float32r  # dtype for the attnV matmul operands (probs, V)

B, S, D = 4, 2048, 768
H, DK = 12, 64
HG = 6            # heads per core (head group)
NP = 3            # head pairs per core
NCH = D // 128    # 6 contraction chunks over d_model
SCH = 4           # seq chunks of 512
QC = 4            # q chunks of 512
KT = S // 128     # 16 key tiles


def build_nc(has_qkv_bias: bool):
    nc = bacc.Bacc("TRN2")
    xT = nc.dram_tensor("xT", [D, S], MM_DT, kind="ExternalInput")
    wqk = nc.dram_tensor("wqk", [D, NP * 2 * 128], MM_DT, kind="ExternalInput")
    wv = nc.dram_tensor("wv", [D, HG * DK], MM_DT, kind="ExternalInput")
    wo = nc.dram_tensor("wo", [HG * DK, D], MM_DT, kind="ExternalInput")
    if has_qkv_bias:
        bqk = nc.dram_tensor("bqk", [128, NP * 2], F32, kind="ExternalInput")
        bv = nc.dram_tensor("bv", [128, HG * DK], F32, kind="ExternalInput")
    out = nc.dram_tensor("out", [S, D], F32, kind="ExternalOutput")
    rden_dram = nc.dram_tensor("rden_scratch", [2 * NP, S], F32)

    with tile.TileContext(nc) as tc, ExitStack() as ctx:
        singles = ctx.enter_context(tc.tile_pool(name="singles", bufs=1))
        xpool = ctx.enter_context(tc.tile_pool(name="xpool", bufs=3))
        pTpool = ctx.enter_context(tc.tile_pool(name="pTpool", bufs=5))
        opool = ctx.enter_context(tc.tile_pool(name="opool", bufs=2))
        rpool = ctx.enter_context(tc.tile_pool(name="rpool", bufs=2))
        dpool = ctx.enter_context(tc.tile_pool(name="dpool", bufs=1))
        ps_sc = ctx.enter_context(tc.tile_pool(name="ps_sc", bufs=2, space="PSUM"))
        ps_cpA = ctx.enter_context(tc.tile_pool(name="ps_cpA", bufs=1, space="PSUM"))
        ps_cpB = ctx.enter_context(tc.tile_pool(name="ps_cpB", bufs=1, space="PSUM"))

        # ---- static weights ----
        wqk_sb = singles.tile([128, NCH, NP * 2 * 128], MM_DT)
        wqk_r = wqk.rearrange("(c p) n -> p c n", p=128)
        for c in range(NCH):
            nc.sync.dma_start(out=wqk_sb[:, c, :], in_=wqk_r[:, c, :])
        wv_sb = singles.tile([128, NCH, HG * DK], MM_DT)
        wo_sb = singles.tile([128, NP, D], MM_DT)
        if has_qkv_bias:
            bqk_sb = singles.tile([128, NP * 2], F32)
            nc.sync.dma_start(out=bqk_sb, in_=bqk[:, :])
            bv_sb = singles.tile([128, HG * DK], F32)
            nc.sync.dma_start(out=bv_sb, in_=bv[:, :])

        # persistent activations
        qkT_sb = singles.tile([128, NP, 2, S], MM_DT)       # [.., pair, q/k, seq]
        vaug_sb = singles.tile([128, KT, HG, DK + 1], ATTN_DT)  # [V_h | ones]
        ctx_sb = singles.tile([128, NP, S], MM_DT)          # pair-stacked context^T
        ones_f = singles.tile([128, 1], F32)
        nc.vector.memset(ones_f, 1.0)
        nc.vector.tensor_copy(
            vaug_sb[:, :, :, DK:DK + 1],
            ones_f[:, None, None, :].to_broadcast((128, KT, HG, 1)),
        )

        xT_r = xT.rearrange("(c p) n -> p c n", p=128)

        # K=1 broadcast stationary (ones row)
        ones_row = singles.tile([1, 64], MM_DT)
        ones_row_f = singles.tile([1, 64], F32)
        nc.vector.memset(ones_row_f, 1.0)
        nc.vector.tensor_copy(ones_row, ones_row_f)

        def emit_qk_proj(xt, s, qk, p3):
            col = (p3 * 2 + qk) * 128
            slot = ps_sc.tile([128, 2, 512], F32, tag="sc", name=f"pj{s}{qk}{p3}")
            ps = slot[:, 0, :]
            for c in range(NCH):
                nc.tensor.matmul(
                    ps, wqk_sb[:, c, col:col + 128], xt[:, c, :],
                    start=(c == 0), stop=(c == NCH - 1),
                )
            dst = qkT_sb[:, p3, qk, s * 512:(s + 1) * 512]
            if has_qkv_bias:
                bias_col = p3 * 2 + qk
                nc.vector.tensor_tensor(
                    dst, ps,
                    bqk_sb[:, bias_col:bias_col + 1].to_broadcast((128, 512)),
                    mybir.AluOpType.add,
                )
            else:
                nc.vector.tensor_copy(dst, ps)

        def emit_v_proj(xt, s, t):
            kt = s * 4 + t
            slot = ps_sc.tile([128, 2, 512], F32, tag="sc", name=f"pv{s}{t}")
            psv = slot[:, 0, 0:HG * DK]
            for c in range(NCH):
                nc.tensor.matmul(
                    psv, xt[:, c, t * 128:(t + 1) * 128], wv_sb[:, c, :],
                    start=(c == 0), stop=(c == NCH - 1),
                )
            vdst = vaug_sb[:, kt, :, 0:DK]
            if has_qkv_bias:
                nc.vector.tensor_tensor(
                    vdst, psv.rearrange("p (h d) -> p h d", h=HG),
                    bv_sb.rearrange("p (h d) -> p h d", h=HG),
                    mybir.AluOpType.add,
                )
            else:
                nc.vector.tensor_copy(
                    vdst, psv.rearrange("p (h d) -> p h d", h=HG))

        def emit_attn_ktile(p3, qc, ktile, ctx_ps):
            qs = qc * 512
            sc = ps_sc.tile([128, 2, 512], F32, tag="sc", name=f"sc{p3}{qc}{ktile}")
            pT = pTpool.tile([128, 2, 512], ATTN_DT, tag="pT")
            for h2 in range(2):
                base = h2 * 64
                nc.tensor.matmul(
                    sc[:, h2, :],
                    qkT_sb[base:base + 64, p3, 1,
                           ktile * 128:(ktile + 1) * 128],
                    qkT_sb[base:base + 64, p3, 0, qs:qs + 512],
                    start=True, stop=True,
                    tile_position=(base, 0),
                )
            nc.scalar.activation(pT, sc, AF.Exp, scale=0.125)
            for h2 in range(2):
                head = p3 * 2 + h2
                nc.tensor.matmul(
                    ctx_ps[h2][0:65, :],
                    vaug_sb[:, ktile, head, :],
                    pT[:, h2, :],
                    start=(ktile == 0), stop=(ktile == KT - 1),
                    skip_group_check=True,
                )

        def alloc_ctx(p3, qc):
            slot_a = ps_cpA.tile([128, 2, 512], F32, tag="cp", name=f"cxa{p3}{qc}")
            slot_b = ps_cpB.tile([128, 2, 512], F32, tag="cp", name=f"cxb{p3}{qc}")
            return [slot_a[:, 0, :], slot_b[:, 0, :]]

        def emit_norm(p3, qc, ctx_ps):
            qs = qc * 512
            nc.vector.tensor_copy(ctx_sb[0:64, p3, qs:qs + 512],
                                  ctx_ps[0][0:64, :])
            nc.vector.tensor_copy(ctx_sb[64:128, p3, qs:qs + 512],
                                  ctx_ps[1][0:64, :])
            da = dpool.tile([1, 512], F32, tag="dtmp_a")
            db = dpool.tile([1, 512], F32, tag="dtmp_b")
            nc.vector.tensor_copy(da, ctx_ps[0][64:65, :])
            nc.vector.tensor_copy(db, ctx_ps[1][64:65, :])
            nc.sync.dma_start(out=rden_dram[2 * p3:2 * p3 + 1, qs:qs + 512],
                              in_=da)
            nc.sync.dma_start(out=rden_dram[2 * p3 + 1:2 * p3 + 2, qs:qs + 512],
                              in_=db)

        def emit_norm2(p3, qc):
            qs = qc * 512
            rbc = rpool.tile([128, 512], F32, tag="rbc")
            for h2 in range(2):
                row = rden_dram[2 * p3 + h2:2 * p3 + h2 + 1, qs:qs + 512]
                bcast = bass.AP(tensor=row.tensor, offset=row.offset,
                                ap=[[0, 64]] + row.ap[1:])
                nc.sync.dma_start(out=rbc[h2 * 64:(h2 + 1) * 64, :], in_=bcast)
            nc.vector.reciprocal_approx_fast(rbc, rbc)
            nc.vector.tensor_tensor(
                ctx_sb[:, p3, qs:qs + 512],
                ctx_sb[:, p3, qs:qs + 512],
                rbc, mybir.AluOpType.mult,
            )

        def emit_outproj(qc):
            for qt in range(4):
                qtg = qc * 4 + qt
                po_pool = ps_cpA if qt % 2 == 0 else ps_cpB
                po = po_pool.tile([128, 2, 512], F32, tag="cp", name=f"po{qtg}")
                for p3 in range(NP):
                    lhsT = ctx_sb[:, p3, qtg * 128:(qtg + 1) * 128]
                    nc.tensor.matmul(
                        po[:, 0, :], lhsT, wo_sb[:, p3, 0:512],
                        start=(p3 == 0), stop=(p3 == NP - 1),
                    )
                    nc.tensor.matmul(
                        po[:, 1, 0:256], lhsT, wo_sb[:, p3, 512:768],
                        start=(p3 == 0), stop=(p3 == NP - 1),
                    )
                ot = opool.tile([128, D], F32, tag="ot")
                nc.vector.tensor_copy(ot[:, 0:512], po[:, 0, :])
                nc.vector.tensor_copy(ot[:, 512:768], po[:, 1, 0:256])
                nc.sync.dma_start(out=out[qtg * 128:(qtg + 1) * 128, :], in_=ot)

        # PE warm-up: ~4us of dummy matmuls during the input DMA wait so
        # phase A starts at full clock (HAM K=8/8).
        wslot = ps_sc.tile([128, 2, 512], F32, tag="sc", name="warm")
        warm_in = singles.tile([128, 512], MM_DT)
        nc.vector.memset(warm_in.bitcast(F32), 1.0)
        for w in range(10):
            nc.tensor.matmul(wslot[0:64, 0, :], warm_in[:, 0:64],
                             warm_in[:, :],
                             start=True, stop=True, skip_group_check=True)

        # ---- phase A: projections ----
        for s in range(SCH):
            xt = xpool.tile([128, NCH, 512], MM_DT, tag="xt_kv")
            for c in range(NCH):
                nc.sync.dma_start(out=xt[:, c, :],
                                  in_=xT_r[:, c, s * 512:(s + 1) * 512])
            if s == 0:
                # deferred weight loads: needed only from the V-projection /
                # out-projection onwards, so they queue behind the first x
                # chunk instead of delaying the first K-projection matmul
                nc.sync.dma_start(out=wv_sb,
                                  in_=wv.rearrange("(c p) n -> p c n", p=128))
                nc.sync.dma_start(out=wo_sb,
                                  in_=wo.rearrange("(c p) n -> p c n", p=128))
            for p3 in range(NP):
                emit_qk_proj(xt, s, 1, p3)   # K^T
            for p3 in range(NP):
                emit_qk_proj(xt, s, 0, p3)   # Q^T
            for t in range(4):
                emit_v_proj(xt, s, t)
        # ---- phase B ----
        # Out-projection of q-chunk qc-1 is emitted after the first pair of
        # chunk qc so it fills PE slack during the ScalarE-paced attention
        # instead of stalling the pipeline at the chunk boundary.
        pending = None
        for qc in range(QC):
            for p3 in range(NP):
                ctx_ps = alloc_ctx(p3, qc)
                for ktile in range(KT):
                    emit_attn_ktile(p3, qc, ktile, ctx_ps)
                emit_norm(p3, qc, ctx_ps)
                if p3 == 0 and pending is not None:
                    for pp in range(NP):
                        emit_norm2(pp, pending)
                    emit_outproj(pending)
            pending = qc
        for pp in range(NP):
            emit_norm2(pp, pending)
        emit_outproj(pending)

    nc.compile()
    return nc


_cache = {}


def _get_nc(has_qkv_bias: bool):
    if has_qkv_bias not in _cache:
        _cache[has_qkv_bias] = build_nc(has_qkv_bias)
    return _cache[has_qkv_bias]


def _prep_core_inputs(x, W_qkv, b_qkv, W_out, g):
    """Host-side shard prep for head-group g (heads g*HG .. g*HG+HG-1)."""
    heads = [g * HG + j for j in range(HG)]
    # W_qkv columns per head h: [h*192, h*192+64) = Q, +64..128 = K, +128..192 = V
    wqk_cols = []
    for p3 in range(NP):
        hA, hB = heads[2 * p3], heads[2 * p3 + 1]
        for qk in range(2):
            off = qk * DK
            wqk_cols.append(W_qkv[:, hA * 192 + off: hA * 192 + off + DK])
            wqk_cols.append(W_qkv[:, hB * 192 + off: hB * 192 + off + DK])
    wqk = np.ascontiguousarray(np.concatenate(wqk_cols, axis=1), dtype=np.float32)
    wv = np.ascontiguousarray(
        np.concatenate(
            [W_qkv[:, h * 192 + 128: h * 192 + 192] for h in heads], axis=1
        ),
        dtype=np.float32,
    )
    wo = np.ascontiguousarray(
        np.concatenate([W_out[h * DK:(h + 1) * DK, :] for h in heads], axis=0),
        dtype=np.float32,
    )
    ins = {"wqk": wqk, "wv": wv, "wo": wo}
    if b_qkv is not None:
        bqk = np.zeros((128, NP * 2), dtype=np.float32)
        for p3 in range(NP):
            hA, hB = heads[2 * p3], heads[2 * p3 + 1]
            for qk in range(2):
                off = qk * DK
                bqk[0:64, p3 * 2 + qk] = b_qkv[hA * 192 + off: hA * 192 + off + DK]
                bqk[64:128, p3 * 2 + qk] = b_qkv[hB * 192 + off: hB * 192 + off + DK]
        bv_flat = np.concatenate(
            [b_qkv[h * 192 + 128: h * 192 + 192] for h in heads]
        ).astype(np.float32)
        ins["bqk"] = bqk
        ins["bv"] = np.ascontiguousarray(np.tile(bv_flat[None, :], (128, 1)))
    return ins


def kernel(x, W_qkv, b_qkv, W_out, b_out):
    x = np.asarray(x, dtype=np.float32)
    W_qkv = np.asarray(W_qkv, dtype=np.float32)
    b_qkv = np.asarray(b_qkv, dtype=np.float32)
    W_out = np.asarray(W_out, dtype=np.float32)
    b_out = np.asarray(b_out, dtype=np.float32)

    has_bias = bool(np.any(b_qkv))
    nc = _get_nc(has_bias)

    group_ins = [
        _prep_core_inputs(x, W_qkv, b_qkv if has_bias else None, W_out, g)
        for g in range(2)
    ]
    in_maps = []
    for c in range(8):
        b, g = c // 2, c % 2
        m = dict(group_ins[g])
        m["xT"] = np.ascontiguousarray(x[b].T)
        in_maps.append(m)

    res = run_bass_kernel_spmd(nc, in_maps, list(range(8)))
    out = np.empty((B, S, D), dtype=np.float32)
    for b in range(B):
        out[b] = res.results[2 * b]["out"] + res.results[2 * b + 1]["out"] + b_out
    return out


# revision 37
# speedup vs baseline: 1.0336x; 1.0336x over previous
"""Trainium2 Bass kernel for 12-head MHA (B=4, S=2048, D=768), 8 NeuronCores.

Sharding: core c -> (batch b = c//2, head-group g = c%2 of 6 heads).
Each core computes its batch's attention for its 6 heads plus the partial
out-projection; the host sums the two partial outputs per batch and adds b_out.

Device dataflow keeps the sequence axis on the SBUF free dimension everywhere,
so no on-chip transposes are needed:
  K^T / Q^T  : stationary = W columns (head-pair packed), moving = x^T chunks
  V          : stationary = x^T chunks, moving = W_v columns (natural layout)
  scores^T   : stationary = K^T tile, moving = Q^T cols (two heads row-tiled)
  exp        : ScalarE from PSUM, one [128, 2, 512] instruction per key tile
  attnV+den  : stationary = [V_h | ones] (M=65), moving = exp'd probs
  out proj   : stationary = pair-stacked context^T, moving = W_out rows

Matmuls run in float32r (TF32-class, 1 cycle/row at N>=256); ATTN_DT can drop
the attnV pair (probs, V) to bf16 for more speed at ~5x the error.

The schedule is software-pipelined: K/V projections first, then per q-chunk
Q projection -> attention -> normalization -> partial out-projection, so
TensorE and ScalarE overlap across stages.
"""

import sys

sys.path.insert(0, "/opt/trn_rl_repo")

from contextlib import ExitStack

import numpy as np

import concourse.bacc as bacc
import concourse.bass as bass
import concourse.tile as tile
from concourse import mybir
from concourse.bass_utils import run_bass_kernel_spmd

# If BASS_TRACE is set in the environment, run_bass_kernel_spmd imports
# antenv.axon_hooks, which is absent in this image. Register a stub so the
# call degrades to "no trace" instead of crashing.
try:
    import antenv.axon_hooks  # noqa: F401
except ImportError:
    import types as _types

    _stub = _types.ModuleType("antenv.axon_hooks")
    _stub.get_axon_ntff_profile_hook = lambda: None
    _stub.set_axon_ntff_profile_hook = lambda h: None
    sys.modules["antenv.axon_hooks"] = _stub

F32 = mybir.dt.float32
BF16 = mybir.dt.bfloat16
AF = mybir.ActivationFunctionType

MM_DT = mybir.dt.float32r   # dtype for projection / scores / out-proj matmuls
ATTN_DT = mybir.dt.float32r  # dtype for the attnV matmul operands (probs, V)

B, S, D = 4, 2048, 768
H, DK = 12, 64
HG = 6            # heads per core (head group)
NP = 3            # head pairs per core
NCH = D // 128    # 6 contraction chunks over d_model
SCH = 4           # seq chunks of 512
QC = 4            # q chunks of 512
KT = S // 128     # 16 key tiles


def build_nc(has_qkv_bias: bool):
    nc = bacc.Bacc("TRN2")
    xT = nc.dram_tensor("xT", [D, S], MM_DT, kind="ExternalInput")
    wqk = nc.dram_tensor("wqk", [D, NP * 2 * 128], MM_DT, kind="ExternalInput")
    wv = nc.dram_tensor("wv", [D, HG * DK], MM_DT, kind="ExternalInput")
    wo = nc.dram_tensor("wo", [HG * DK, D], MM_DT, kind="ExternalInput")
    if has_qkv_bias:
        bqk = nc.dram_tensor("bqk", [128, NP * 2], F32, kind="ExternalInput")
        bv = nc.dram_tensor("bv", [128, HG * DK], F32, kind="ExternalInput")
    out = nc.dram_tensor("out", [S, D], F32, kind="ExternalOutput")
    rden_dram = nc.dram_tensor("rden_scratch", [2 * NP, S], F32)

    with tile.TileContext(nc) as tc, ExitStack() as ctx:
        singles = ctx.enter_context(tc.tile_pool(name="singles", bufs=1))
        xpool = ctx.enter_context(tc.tile_pool(name="xpool", bufs=2))
        pTpool = ctx.enter_context(tc.tile_pool(name="pTpool", bufs=5))
        opool = ctx.enter_context(tc.tile_pool(name="opool", bufs=2))
        rpool = ctx.enter_context(tc.tile_pool(name="rpool", bufs=2))
        dpool = ctx.enter_context(tc.tile_pool(name="dpool", bufs=1))
        ps_sc = ctx.enter_context(tc.tile_pool(name="ps_sc", bufs=2, space="PSUM"))
        ps_cpA = ctx.enter_context(tc.tile_pool(name="ps_cpA", bufs=1, space="PSUM"))
        ps_cpB = ctx.enter_context(tc.tile_pool(name="ps_cpB", bufs=1, space="PSUM"))

        # ---- static weights ----
        wqk_sb = singles.tile([128, NCH, NP * 2 * 128], MM_DT)
        wqk_r = wqk.rearrange("(c p) n -> p c n", p=128)
        for c in range(NCH):
            nc.sync.dma_start(out=wqk_sb[:, c, :], in_=wqk_r[:, c, :])
        wv_sb = singles.tile([128, NCH, HG * DK], MM_DT)
        wo_sb = singles.tile([128, NP, D], MM_DT)
        if has_qkv_bias:
            bqk_sb = singles.tile([128, NP * 2], F32)
            nc.sync.dma_start(out=bqk_sb, in_=bqk[:, :])
            bv_sb = singles.tile([128, HG * DK], F32)
            nc.sync.dma_start(out=bv_sb, in_=bv[:, :])

        # persistent activations
        qkT_sb = singles.tile([128, NP, 2, S], MM_DT)       # [.., pair, q/k, seq]
        vaug_sb = singles.tile([128, KT, HG, DK + 1], ATTN_DT)  # [V_h | ones]
        ctx_sb = singles.tile([128, NP, S], MM_DT)          # pair-stacked context^T
        ones_f = singles.tile([128, 1], F32)
        nc.vector.memset(ones_f, 1.0)
        nc.vector.tensor_copy(
            vaug_sb[:, :, :, DK:DK + 1],
            ones_f[:, None, None, :].to_broadcast((128, KT, HG, 1)),
        )

        xT_r = xT.rearrange("(c p) n -> p c n", p=128)

        # K=1 broadcast stationary (ones row)
        ones_row = singles.tile([1, 64], MM_DT)
        ones_row_f = singles.tile([1, 64], F32)
        nc.vector.memset(ones_row_f, 1.0)
        nc.vector.tensor_copy(ones_row, ones_row_f)

        def emit_qk_proj(xt, s, qk, p3):
            col = (p3 * 2 + qk) * 128
            slot = ps_sc.tile([128, 2, 512], F32, tag="sc", name=f"pj{s}{qk}{p3}")
            ps = slot[:, 0, :]
            for c in range(NCH):
                nc.tensor.matmul(
                    ps, wqk_sb[:, c, col:col + 128], xt[:, c, :],
                    start=(c == 0), stop=(c == NCH - 1),
                )
            dst = qkT_sb[:, p3, qk, s * 512:(s + 1) * 512]
            if has_qkv_bias:
                bias_col = p3 * 2 + qk
                nc.vector.tensor_tensor(
                    dst, ps,
                    bqk_sb[:, bias_col:bias_col + 1].to_broadcast((128, 512)),
                    mybir.AluOpType.add,
                )
            else:
                nc.vector.tensor_copy(dst, ps)

        def emit_v_proj(xt, s, t):
            kt = s * 4 + t
            slot = ps_sc.tile([128, 2, 512], F32, tag="sc", name=f"pv{s}{t}")
            psv = slot[:, 0, 0:HG * DK]
            for c in range(NCH):
                nc.tensor.matmul(
                    psv, xt[:, c, t * 128:(t + 1) * 128], wv_sb[:, c, :],
                    start=(c == 0), stop=(c == NCH - 1),
                )
            vdst = vaug_sb[:, kt, :, 0:DK]
            if has_qkv_bias:
                nc.vector.tensor_tensor(
                    vdst, psv.rearrange("p (h d) -> p h d", h=HG),
                    bv_sb.rearrange("p (h d) -> p h d", h=HG),
                    mybir.AluOpType.add,
                )
            else:
                nc.vector.tensor_copy(
                    vdst, psv.rearrange("p (h d) -> p h d", h=HG))

        def emit_attn_ktile(p3, qc, ktile, ctx_ps):
            qs = qc * 512
            sc = ps_sc.tile([128, 2, 512], F32, tag="sc", name=f"sc{p3}{qc}{ktile}")
            pT = pTpool.tile([128, 2, 512], ATTN_DT, tag="pT")
            for h2 in range(2):
                base = h2 * 64
                nc.tensor.matmul(
                    sc[:, h2, :],
                    qkT_sb[base:base + 64, p3, 1,
                           ktile * 128:(ktile + 1) * 128],
                    qkT_sb[base:base + 64, p3, 0, qs:qs + 512],
                    start=True, stop=True,
                    tile_position=(base, 0),
                )
            nc.scalar.activation(pT, sc, AF.Exp, scale=0.125)
            for h2 in range(2):
                head = p3 * 2 + h2
                nc.tensor.matmul(
                    ctx_ps[h2][0:65, :],
                    vaug_sb[:, ktile, head, :],
                    pT[:, h2, :],
                    start=(ktile == 0), stop=(ktile == KT - 1),
                    skip_group_check=True,
                )

        def alloc_ctx(p3, qc):
            slot_a = ps_cpA.tile([128, 2, 512], F32, tag="cp", name=f"cxa{p3}{qc}")
            slot_b = ps_cpB.tile([128, 2, 512], F32, tag="cp", name=f"cxb{p3}{qc}")
            return [slot_a[:, 0, :], slot_b[:, 0, :]]

        def emit_norm(p3, qc, ctx_ps):
            qs = qc * 512
            nc.vector.tensor_copy(ctx_sb[0:64, p3, qs:qs + 512],
                                  ctx_ps[0][0:64, :])
            nc.vector.tensor_copy(ctx_sb[64:128, p3, qs:qs + 512],
                                  ctx_ps[1][0:64, :])
            da = dpool.tile([1, 512], F32, tag="dtmp_a")
            db = dpool.tile([1, 512], F32, tag="dtmp_b")
            nc.vector.tensor_copy(da, ctx_ps[0][64:65, :])
            nc.vector.tensor_copy(db, ctx_ps[1][64:65, :])
            nc.sync.dma_start(out=rden_dram[2 * p3:2 * p3 + 1, qs:qs + 512],
                              in_=da)
            nc.sync.dma_start(out=rden_dram[2 * p3 + 1:2 * p3 + 2, qs:qs + 512],
                              in_=db)

        def emit_norm2(p3, qc):
            qs = qc * 512
            rbc = rpool.tile([128, 512], F32, tag="rbc")
            for h2 in range(2):
                row = rden_dram[2 * p3 + h2:2 * p3 + h2 + 1, qs:qs + 512]
                bcast = bass.AP(tensor=row.tensor, offset=row.offset,
                                ap=[[0, 64]] + row.ap[1:])
                nc.sync.dma_start(out=rbc[h2 * 64:(h2 + 1) * 64, :], in_=bcast)
            nc.vector.reciprocal_approx_fast(rbc, rbc)
            nc.vector.tensor_tensor(
                ctx_sb[:, p3, qs:qs + 512],
                ctx_sb[:, p3, qs:qs + 512],
                rbc, mybir.AluOpType.mult,
            )

        def emit_outproj(qc):
            for qt in range(4):
                qtg = qc * 4 + qt
                po_pool = ps_cpA if qt % 2 == 0 else ps_cpB
                po = po_pool.tile([128, 2, 512], F32, tag="cp", name=f"po{qtg}")
                for p3 in range(NP):
                    lhsT = ctx_sb[:, p3, qtg * 128:(qtg + 1) * 128]
                    nc.tensor.matmul(
                        po[:, 0, :], lhsT, wo_sb[:, p3, 0:512],
                        start=(p3 == 0), stop=(p3 == NP - 1),
                    )
                    nc.tensor.matmul(
                        po[:, 1, 0:256], lhsT, wo_sb[:, p3, 512:768],
                        start=(p3 == 0), stop=(p3 == NP - 1),
                    )
                ot = opool.tile([128, D], F32, tag="ot")
                nc.vector.tensor_copy(ot[:, 0:512], po[:, 0, :])
                nc.vector.tensor_copy(ot[:, 512:768], po[:, 1, 0:256])
                nc.sync.dma_start(out=out[qtg * 128:(qtg + 1) * 128, :], in_=ot)

        # PE warm-up: ~4us of dummy matmuls during the input DMA wait so
        # phase A starts at full clock (HAM K=8/8).
        wslot = ps_sc.tile([128, 2, 512], F32, tag="sc", name="warm")
        warm_in = singles.tile([128, 512], MM_DT)
        nc.vector.memset(warm_in.bitcast(F32), 1.0)
        for w in range(10):
            nc.tensor.matmul(wslot[0:64, 0, :], warm_in[:, 0:64],
                             warm_in[:, :],
                             start=True, stop=True, skip_group_check=True)

        # ---- phase A: projections ----
        for s in range(SCH):
            xt = xpool.tile([128, NCH, 512], MM_DT, tag="xt_kv")
            for c in range(NCH):
                nc.sync.dma_start(out=xt[:, c, :],
                                  in_=xT_r[:, c, s * 512:(s + 1) * 512])
            if s == 0:
                # deferred weight loads: needed only from the V-projection /
                # out-projection onwards, so they queue behind the first x
                # chunk instead of delaying the first K-projection matmul
                nc.sync.dma_start(out=wv_sb,
                                  in_=wv.rearrange("(c p) n -> p c n", p=128))
                nc.sync.dma_start(out=wo_sb,
                                  in_=wo.rearrange("(c p) n -> p c n", p=128))
            for p3 in range(NP):
                emit_qk_proj(xt, s, 1, p3)   # K^T
            for p3 in range(NP):
                emit_qk_proj(xt, s, 0, p3)   # Q^T
            for t in range(4):
                emit_v_proj(xt, s, t)
        # ---- phase B ----
        # Out-projection of q-chunk qc-1 is emitted after the first pair of
        # chunk qc so it fills PE slack during the ScalarE-paced attention
        # instead of stalling the pipeline at the chunk boundary.
        pending = None
        for qc in range(QC):
            for p3 in range(NP):
                ctx_ps = alloc_ctx(p3, qc)
                for ktile in range(KT):
                    emit_attn_ktile(p3, qc, ktile, ctx_ps)
                emit_norm(p3, qc, ctx_ps)
                if p3 == 0 and pending is not None:
                    for pp in range(NP):
                        emit_norm2(pp, pending)
                    emit_outproj(pending)
            pending = qc
        for pp in range(NP):
            emit_norm2(pp, pending)
        emit_outproj(pending)

    nc.compile()
    return nc


_cache = {}


def _get_nc(has_qkv_bias: bool):
    if has_qkv_bias not in _cache:
        _cache[has_qkv_bias] = build_nc(has_qkv_bias)
    return _cache[has_qkv_bias]


def _prep_core_inputs(x, W_qkv, b_qkv, W_out, g):
    """Host-side shard prep for head-group g (heads g*HG .. g*HG+HG-1)."""
    heads = [g * HG + j for j in range(HG)]
    # W_qkv columns per head h: [h*192, h*192+64) = Q, +64..128 = K, +128..192 = V
    wqk_cols = []
    for p3 in range(NP):
        hA, hB = heads[2 * p3], heads[2 * p3 + 1]
        for qk in range(2):
            off = qk * DK
            wqk_cols.append(W_qkv[:, hA * 192 + off: hA * 192 + off + DK])
            wqk_cols.append(W_qkv[:, hB * 192 + off: hB * 192 + off + DK])
    wqk = np.ascontiguousarray(np.concatenate(wqk_cols, axis=1), dtype=np.float32)
    wv = np.ascontiguousarray(
        np.concatenate(
            [W_qkv[:, h * 192 + 128: h * 192 + 192] for h in heads], axis=1
        ),
        dtype=np.float32,
    )
    wo = np.ascontiguousarray(
        np.concatenate([W_out[h * DK:(h + 1) * DK, :] for h in heads], axis=0),
        dtype=np.float32,
    )
    ins = {"wqk": wqk, "wv": wv, "wo": wo}
    if b_qkv is not None:
        bqk = np.zeros((128, NP * 2), dtype=np.float32)
        for p3 in range(NP):
            hA, hB = heads[2 * p3], heads[2 * p3 + 1]
            for qk in range(2):
                off = qk * DK
                bqk[0:64, p3 * 2 + qk] = b_qkv[hA * 192 + off: hA * 192 + off + DK]
                bqk[64:128, p3 * 2 + qk] = b_qkv[hB * 192 + off: hB * 192 + off + DK]
        bv_flat = np.concatenate(
            [b_qkv[h * 192 + 128: h * 192 + 192] for h in heads]
        ).astype(np.float32)
        ins["bqk"] = bqk
        ins["bv"] = np.ascontiguousarray(np.tile(bv_flat[None, :], (128, 1)))
    return ins


def kernel(x, W_qkv, b_qkv, W_out, b_out):
    x = np.asarray(x, dtype=np.float32)
    W_qkv = np.asarray(W_qkv, dtype=np.float32)
    b_qkv = np.asarray(b_qkv, dtype=np.float32)
    W_out = np.asarray(W_out, dtype=np.float32)
    b_out = np.asarray(b_out, dtype=np.float32)

    has_bias = bool(np.any(b_qkv))
    nc = _get_nc(has_bias)

    group_ins = [
        _prep_core_inputs(x, W_qkv, b_qkv if has_bias else None, W_out, g)
        for g in range(2)
    ]
    in_maps = []
    for c in range(8):
        b, g = c // 2, c % 2
        m = dict(group_ins[g])
        m["xT"] = np.ascontiguousarray(x[b].T)
        in_maps.append(m)

    res = run_bass_kernel_spmd(nc, in_maps, list(range(8)))
    out = np.empty((B, S, D), dtype=np.float32)
    for b in range(B):
        out[b] = res.results[2 * b]["out"] + res.results[2 * b + 1]["out"] + b_out
    return out
